# revision 12
# baseline (speedup 1.0000x reference)
"""NeuralCDE RK4 solver as a Bass/Tile kernel on 8 Trainium2 cores.

Data-parallel over batch: B=1024 -> 128 rows per core (one partition tile).
The 127-step RK4 scan is fully unrolled. Critical path per stage:
    mm1acc (PE) : h_ps[:, s*128:] += W1zH.T @ (alpha*k)   (fp16)
    relu  (DVE) : hS = relu(h_ps_slice + bias1(t))
    mm2   (PE)  : f_ps[128b,512] = hS.T @ W2  (c-major cols)
    tanh  (ACT) : fS = tanh(f_psum)
    mul   (DVE) : u = fS * g_exp   (flat [128,512] fp16, 2x mode)
    tree  (DVE) : 3 halving adds -> kn[128b,64h] fp16  (vs 1x tensor_reduce)
    T     (PE)  : kn^T -> ksP/accP PSUM (fp16 in, 1-pass)
    kh    (DVE) : alpha*kT -> SBUF fp16
Big mm1 (W1z.T @ zT) per stage is pre-issued off-chain (h = W1^T z + a W1^T k).
Zero-weight filler matmuls keep the PE HAM clock gate at K=8/8 (2.4 GHz);
without them every matmul runs at the cold 1.2 GHz rate.
State z^T lives in one big SBUF buffer [64, 128*128] fp32; slots stream out
to DRAM as they finish.
"""

import numpy as np
import ml_dtypes

import concourse.bacc as bacc
import concourse.bass as bass
import concourse.mybir as mybir
from concourse.tile import TileContext
from concourse.bass_utils import run_bass_kernel_spmd

F32 = mybir.dt.float32
F32R = mybir.dt.float32r
BF16 = mybir.dt.bfloat16
FP16 = mybir.dt.float16
B = 1024
L = 128
C_IN = 8
HID = 64
MLP_H = 128
INIT_H = 20
NSTEP = L - 1  # 127
NCORES = 8
BL = B // NCORES  # 128 batch rows per core
NF = HID * C_IN  # 512

_CACHE: dict = {}


def _flags():
    import os
    return (
        int(os.environ.get("F_POST", "6")),   # fillers after mm2 (tanh+mul+tree gap)
        int(os.environ.get("F_KH", "1")),     # fillers after transposes (kh gap)
        int(os.environ.get("F_RELU", "1")),   # fillers after mm1acc (relu gap)
        int(os.environ.get("F_WARMUP", "48")),
        int(os.environ.get("T_FP16", "1")),
    )


def _build(nstep: int, with_b2: bool):
    import time as _time

    f_post, f_kh, f_relu, f_warmup, t_fp16 = _flags()
    TD = FP16 if t_fp16 else F32
    t0 = _time.time()
    nc = bacc.Bacc()
    gx_in = nc.dram_tensor("gexp", [BL, nstep * 3 * NF], FP16, kind="ExternalInput")
    b1_in = nc.dram_tensor("bias1", [MLP_H, nstep * 3], F32, kind="ExternalInput")
    w1z_in = nc.dram_tensor("w1z", [HID, MLP_H], F32R, kind="ExternalInput")
    w1zh_in = nc.dram_tensor("w1zh", [HID, MLP_H], FP16, kind="ExternalInput")
    w2_in = nc.dram_tensor("w2", [MLP_H, NF], FP16, kind="ExternalInput")
    b2_in = nc.dram_tensor("b2r", [1, NF], FP16, kind="ExternalInput")
    ones_in = nc.dram_tensor("onesr", [1, BL], FP16, kind="ExternalInput")
    id_in = nc.dram_tensor("ident", [BL, BL], TD, kind="ExternalInput")
    z0t_in = nc.dram_tensor("z0t", [HID, BL], F32R, kind="ExternalInput")
    zs_out = nc.dram_tensor("zs", [HID, (nstep + 1) * BL], F32, kind="ExternalOutput")

    with TileContext(nc) as tc:
        with (
            tc.tile_pool(name="const", bufs=1) as cp,
            tc.tile_pool(name="zst", bufs=1) as zp,
            tc.tile_pool(name="gx", bufs=2) as gp,
            tc.tile_pool(name="hs", bufs=3) as hp,
            tc.tile_pool(name="fs", bufs=2) as fp,
            tc.tile_pool(name="us", bufs=2) as up,
            tc.tile_pool(name="t1", bufs=2) as t1p,
            tc.tile_pool(name="t2", bufs=2) as t2p,
            tc.tile_pool(name="ks", bufs=3) as kp,
            tc.tile_pool(name="an", bufs=2) as ap,
            tc.tile_pool(name="kh", bufs=2) as khp,
            tc.tile_pool(name="ph", bufs=4, space="PSUM") as ph,
            tc.tile_pool(name="pf", bufs=1, space="PSUM") as pf,
            tc.tile_pool(name="pks", bufs=2, space="PSUM") as pks,
            tc.tile_pool(name="pfill", bufs=1, space="PSUM") as pfill,
        ):
            b1S = cp.tile([MLP_H, nstep * 3], F32)
            w1zS = cp.tile([HID, MLP_H], F32R)
            w1zH = cp.tile([HID, MLP_H], FP16)
            w2S = cp.tile([MLP_H, NF], FP16)
            b2S = cp.tile([1, NF], FP16)
            onesS = cp.tile([1, BL], FP16)
            idS = cp.tile([BL, BL], TD)
            zall = zp.tile([HID, (nstep + 1) * BL], F32R)
            wt = cp.tile([BL, BL], BF16, name="wt")
            nc.vector.memset(wt[:], 0.0)

            nc.sync.dma_start(out=b1S[:], in_=b1_in[:])
            nc.sync.dma_start(out=w1zS[:], in_=w1z_in[:])
            nc.sync.dma_start(out=w1zH[:], in_=w1zh_in[:])
            nc.sync.dma_start(out=w2S[:], in_=w2_in[:])
            nc.sync.dma_start(out=b2S[:], in_=b2_in[:])
            nc.sync.dma_start(out=onesS[:], in_=ones_in[:])
            nc.sync.dma_start(out=idS[:], in_=id_in[:])
            nc.sync.dma_start(out=zall[:, 0:BL], in_=z0t_in[:])
            nc.sync.dma_start(out=zs_out[:, 0:BL], in_=z0t_in[:].bitcast(F32))

            def fill(n):
                for _ in range(n):
                    wp = pfill.tile([32, BL], F32, tag="fl", name="wp")
                    nc.tensor.matmul(
                        wp[:], lhsT=wt[:, 0:32], rhs=wt[:], start=True, stop=True
                    )

            fill(f_warmup)

            # g-expansion slices streamed per step, double buffered
            def g_tile(step):
                gT = gp.tile([BL, 3 * NF], FP16, tag="gx", name="gx")
                nc.sync.dma_start(
                    out=gT[:], in_=gx_in[:, step * 3 * NF : (step + 1) * 3 * NF]
                )
                return gT

            g_cur = g_tile(0)
            g_next = None

            CLS = (0, 1, 1, 2)
            ALPHA = (0.5, 0.25, 0.5, 1.0 / 6.0)
            # kh scale for feeding stage s's mm1acc (see ksrc selection below)
            KH_A = (1.0 / 6.0, 0.5, 0.25, 0.5)

            def hps_tile():
                return ph.tile([MLP_H, BL], F32, tag="hps", name="hps")

            h_tiles = [hps_tile() for _ in range(4)]
            h0_next = None
            prev_ktP = None   # k^T of previous stage (feeds this stage's kh)
            prev_accT = None  # transposed RK4 sum of previous step
            kn_s = None
            acc_nat = None

            for step in range(nstep):
                zT = zall[:, step * BL : (step + 1) * BL]
                zT_prev = zall[:, (step - 1) * BL : step * BL]
                if step == 0:
                    # big mm1 for all 4 stages of step 0 (s=0 has no k term)
                    nc.tensor.matmul(
                        h_tiles[0][:], lhsT=w1zS[:], rhs=zT, start=True, stop=True
                    )
                    for s in range(1, 4):
                        nc.tensor.matmul(
                            h_tiles[s][:], lhsT=w1zS[:], rhs=zT,
                            start=True, stop=False,
                        )
                else:
                    h_tiles = [h0_next, None, None, None]
                if step + 1 < nstep:
                    g_next = g_tile(step + 1)

                for s in range(4):
                    col = step * 3 + CLS[s]
                    has_b = not (step == 0 and s == 0)
                    if has_b:
                        # kh = alpha * k^T  (PSUM fp16 -> SBUF fp16)
                        ksrc = prev_accT if s == 0 else prev_ktP
                        kh = khp.tile([HID, BL], FP16, tag="kh", name="kh")
                        nc.vector.tensor_scalar_mul(kh[:], ksrc[:], KH_A[s])
                        if s == 0 and step > 0:
                            # z update for this grid point; off critical path
                            nc.vector.scalar_tensor_tensor(
                                out=zall[:, step * BL : (step + 1) * BL],
                                in0=prev_accT[:],
                                scalar=1.0 / 6.0,
                                in1=zT_prev,
                                op0=mybir.AluOpType.mult,
                                op1=mybir.AluOpType.add,
                            )
                            nc.sync.dma_start(
                                out=zs_out[:, step * BL : (step + 1) * BL],
                                in_=zall[:, step * BL : (step + 1) * BL].bitcast(F32),
                            )
                        nc.tensor.matmul(
                            h_tiles[s][:], lhsT=w1zH[:], rhs=kh[:],
                            start=False, stop=True,
                        )
                    # big mm1 for the next stage's h tile (off critical path)
                    if s < 3:
                        if step > 0:
                            h_tiles[s + 1] = hps_tile()
                            nc.tensor.matmul(
                                h_tiles[s + 1][:], lhsT=w1zS[:], rhs=zT,
                                start=True, stop=False,
                            )
                    else:
                        if step + 1 < nstep:
                            h0_next = hps_tile()
                            nc.tensor.matmul(
                                h0_next[:], lhsT=w1zS[:], rhs=zT,
                                start=True, stop=False,
                            )
                    fill(f_relu)
                    # relu with per-stage bias (bias1 = b1 + t*W1[0])
                    hS = hp.tile([MLP_H, BL], FP16, tag="hs")
                    nc.vector.tensor_scalar(
                        hS[:],
                        h_tiles[s][:],
                        b1S[:, col : col + 1],
                        0.0,
                        op0=mybir.AluOpType.add,
                        op1=mybir.AluOpType.max,
                    )
                    f_ps = pf.tile([BL, NF], F32, tag="fps")
                    if with_b2:
                        nc.tensor.matmul(
                            f_ps[:], lhsT=onesS[:], rhs=b2S[:], start=True, stop=False
                        )
                    nc.tensor.matmul(
                        f_ps[:], lhsT=hS[:], rhs=w2S[:], start=not with_b2, stop=True
                    )
                    fill(f_post)
                    fS = fp.tile([BL, NF], FP16, tag="fs")
                    nc.scalar.activation(
                        fS[:], f_ps[:], mybir.ActivationFunctionType.Tanh
                    )
                    # u = fS * g  (flat fp16, 2x DVE mode)
                    u = up.tile([BL, NF], FP16, tag="u")
                    gv = g_cur[:, CLS[s] * NF : (CLS[s] + 1) * NF]
                    nc.vector.tensor_tensor(
                        out=u[:], in0=fS[:], in1=gv, op=mybir.AluOpType.mult
                    )
                    # halving add tree over c (c-major layout): 512->256->128->64
                    t1 = t1p.tile([BL, NF // 2], FP16, tag="t1")
                    nc.vector.tensor_tensor(
                        out=t1[:], in0=u[:, 0 : NF // 2], in1=u[:, NF // 2 : NF],
                        op=mybir.AluOpType.add,
                    )
                    t2 = t2p.tile([BL, NF // 4], FP16, tag="t2")
                    nc.vector.tensor_tensor(
                        out=t2[:], in0=t1[:, 0 : NF // 4], in1=t1[:, NF // 4 : NF // 2],
                        op=mybir.AluOpType.add,
                    )
                    kn = kp.tile([BL, HID], TD, tag="kn")
                    nc.vector.tensor_tensor(
                        out=kn[:], in0=t2[:, 0:HID], in1=t2[:, HID : 2 * HID],
                        op=mybir.AluOpType.add,
                    )
                    # RK4 sum accumulated in natural layout on DVE (ping-pong)
                    if s == 0:
                        acc_nat = kn
                    else:
                        acc_new = ap.tile([BL, HID], TD, tag="an", name="an")
                        nc.vector.tensor_tensor(
                            out=acc_new[:], in0=acc_nat[:], in1=kn[:],
                            op=mybir.AluOpType.add,
                        )
                        acc_nat = acc_new
                    # transpose on PE: k^T for next stage (s<3) or acc^T (s=3)
                    ktP = pks.tile([HID, BL], TD, tag="kt", name="kt")
                    nc.tensor.matmul(
                        ktP[:], lhsT=(kn[:] if s < 3 else acc_nat[:]), rhs=idS[:],
                        is_transpose=True, start=True, stop=True,
                    )
                    if s < 3:
                        prev_ktP = ktP
                    else:
                        prev_accT = ktP
                    fill(f_kh)
                g_cur = g_next

            # final z update (last grid point)
            nc.vector.scalar_tensor_tensor(
                out=zall[:, nstep * BL : (nstep + 1) * BL],
                in0=prev_accT[:],
                scalar=1.0 / 6.0,
                in1=zall[:, (nstep - 1) * BL : nstep * BL],
                op0=mybir.AluOpType.mult,
                op1=mybir.AluOpType.add,
            )
            nc.sync.dma_start(
                out=zs_out[:, nstep * BL : (nstep + 1) * BL],
                in_=zall[:, nstep * BL : (nstep + 1) * BL].bitcast(F32),
            )
    import sys

    print(f"[kernel] tile trace+schedule: {_time.time()-t0:.1f}s", file=sys.stderr)
    t1 = _time.time()
    nc.finalize()
    print(f"[kernel] finalize: {_time.time()-t1:.1f}s", file=sys.stderr)
    return nc


def _get_nc(nstep: int, with_b2: bool):
    key = (nstep, with_b2) + _flags()
    if key not in _CACHE:
        _CACHE[key] = _build(nstep, with_b2)
    return _CACHE[key]


def _host_prep(coeffs, Wi1, bi1, Wi2, bi2, W1, b1, W2, b2, nstep: int):
    coeffs = np.asarray(coeffs, dtype=np.float32)
    a = coeffs[:, :, 0:8]
    b = coeffs[:, :, 8:16]
    c = coeffs[:, :, 16:24]
    d = coeffs[:, :, 24:32]

    X0 = a[:, 0]
    z0 = np.tanh(
        np.maximum(X0 @ Wi1 + bi1, 0.0).astype(np.float32) @ Wi2 + bi2
    ).astype(np.float32)

    g = np.empty((B, nstep, 3, C_IN), dtype=np.float32)
    g[:, :, 0] = b[:, :nstep]
    g[:, :, 1] = 2.0 * b[:, :nstep] + 2.0 * c[:, :nstep] + 1.5 * d[:, :nstep]
    # stage-4 derivative: dXdt at t=i+1
    last = NSTEP - 1  # 126 in full problem
    for i in range(nstep):
        if i < last:
            g[:, i, 2] = b[:, i + 1]
        else:
            g[:, i, 2] = b[:, i] + 2.0 * c[:, i] + 3.0 * d[:, i]
    # expand to c-major 512 wide: gexp[b, i, cls, c*64+h] = g[b, i, cls, c]
    gexp = np.broadcast_to(
        g.astype(np.float16)[:, :, :, :, None], (B, nstep, 3, C_IN, HID)
    ).reshape(B, nstep * 3 * NF)

    tcols = np.empty((nstep, 3), dtype=np.float32)
    tcols[:, 0] = np.arange(nstep, dtype=np.float32)
    tcols[:, 1] = tcols[:, 0] + 0.5
    tcols[:, 2] = tcols[:, 0] + 1.0
    # bias1[m, step*3+cls] = b1[m] + t * W1[0, m]
    bias1 = (
        b1[None, None, :] + tcols[:, :, None] * W1[0][None, None, :]
    ).astype(np.float32)
    bias1 = bias1.reshape(nstep * 3, MLP_H).T.copy()  # [128, nstep*3]

    # w2 columns reordered c-major: col (c*64+h) <- h*8+c
    perm = np.arange(NF).reshape(HID, C_IN).T.reshape(-1)  # [c*64+h] -> h*8+c
    w2cm = np.ascontiguousarray(W2[:, perm], dtype=np.float16)
    b2cm = np.ascontiguousarray(b2[perm][None, :], dtype=np.float16)

    shared = {
        "bias1": bias1,
        "w1z": np.ascontiguousarray(W1[1:], dtype=np.float32),
        "w1zh": np.ascontiguousarray(W1[1:], dtype=np.float16),
        "w2": w2cm,
        "b2r": b2cm,
        "onesr": np.ones((1, BL), dtype=np.float16),
        "ident": np.eye(BL, dtype=(np.float16 if _flags()[4] else np.float32)),
    }
    in_maps = []
    for core in range(NCORES):
        sl = slice(core * BL, (core + 1) * BL)
        m = dict(shared)
        m["gexp"] = np.ascontiguousarray(gexp[sl])
        m["z0t"] = np.ascontiguousarray(z0[sl].T)
        in_maps.append(m)
    return in_maps, z0


def kernel(coeffs, Wi1, bi1, Wi2, bi2, W1, b1, W2, b2, _nstep: int = NSTEP,
           _trace: bool = False):
    import time as _time
    import sys

    nstep = _nstep
    with_b2 = bool(np.any(np.asarray(b2)))
    nc = _get_nc(nstep, with_b2)
    in_maps, _ = _host_prep(
        coeffs, Wi1, bi1, Wi2, bi2, W1, b1, W2, b2, nstep
    )
    t0 = _time.time()
    res = run_bass_kernel_spmd(nc, in_maps, list(range(NCORES)), trace=_trace)
    print(f"[kernel] spmd run (compile+exec): {_time.time()-t0:.1f}s", file=sys.stderr)
    out = np.empty((B, nstep + 1, HID), dtype=np.float32)
    for core in range(NCORES):
        zs = res.results[core]["zs"].reshape(HID, nstep + 1, BL)
        out[core * BL : (core + 1) * BL] = zs.transpose(2, 1, 0)
    if _trace:
        kernel.last_results = res
    return out


# revision 13
# speedup vs baseline: 1.0165x; 1.0165x over previous
"""NeuralCDE RK4 solver as a Bass/Tile kernel on 8 Trainium2 cores.

Data-parallel over batch: B=1024 -> 128 rows per core, processed as TWO
interleaved streams of 64 rows. The per-stage dependency chain is dominated
by free-dim-bound ops (mm2 512 cols, tanh/mul/reduce over 512 elems) whose
cost is independent of batch rows; each engine is <50% busy on one stream,
so two independent RK4 chains overlap for ~2x throughput.

Per stage and stream (critical path):
    kh    (DVE) : alpha*k^T PSUM->SBUF fp16
    mm1acc (PE) : h_ps slot += W1zH.T @ kh          (big W1z.T @ zT pre-issued)
    relu  (ACT) : hS = relu(h_ps + bias1(t))        (bias fused, frees DVE)
    mm2   (PE)  : f_ps[64b,512] = hS.T @ W2
    tanh  (ACT) : fS = tanh(f_psum)
    mul   (DVE) : u = fS * g (broadcast over h)
    red   (DVE) : kn[64b,64h] = sum_c u
    T     (PE)  : kn^T -> kt slot (fp16 1-pass); RK4 sum via DVE ping-pong adds
State z^T lives in one big SBUF buffer [64, 128*128] fp32r; slots stream out
to DRAM as they finish. PE runs at the cold 1.2 GHz HAM rate (filler matmuls
do not lift the clock gate in this environment - verified).
"""

import numpy as np
import ml_dtypes

import concourse.bacc as bacc
import concourse.bass as bass
import concourse.mybir as mybir
from concourse.tile import TileContext
from concourse.bass_utils import run_bass_kernel_spmd

F32 = mybir.dt.float32
F32R = mybir.dt.float32r
BF16 = mybir.dt.bfloat16
FP16 = mybir.dt.float16
B = 1024
L = 128
C_IN = 8
HID = 64
MLP_H = 128
INIT_H = 20
NSTEP = L - 1  # 127
NCORES = 8
BL = B // NCORES  # 128 batch rows per core
BS = BL // 2  # 64 rows per stream
NF = HID * C_IN  # 512

_CACHE: dict = {}


def _flags():
    import os
    return (
        int(os.environ.get("F16_BIG", "0")),  # big mm1 in fp16 instead of f32r
        int(os.environ.get("R_ACT", "1")),    # relu on ACT engine (else DVE)
    )


def _build(nstep: int, with_b2: bool):
    import time as _time

    f16_big, r_act = _flags()
    BD = FP16 if f16_big else F32R
    t0 = _time.time()
    nc = bacc.Bacc()
    gA_in = nc.dram_tensor("gA", [BS, nstep * 3 * C_IN], FP16, kind="ExternalInput")
    gB_in = nc.dram_tensor("gB", [BS, nstep * 3 * C_IN], FP16, kind="ExternalInput")
    b1_in = nc.dram_tensor("bias1", [MLP_H, nstep * 3], F32, kind="ExternalInput")
    w1z_in = nc.dram_tensor("w1z", [HID, MLP_H], BD, kind="ExternalInput")
    w1zh_in = nc.dram_tensor("w1zh", [HID, MLP_H], FP16, kind="ExternalInput")
    w2_in = nc.dram_tensor("w2", [MLP_H, NF], FP16, kind="ExternalInput")
    b2_in = nc.dram_tensor("b2r", [1, NF], FP16, kind="ExternalInput")
    ones_in = nc.dram_tensor("onesr", [1, BS], FP16, kind="ExternalInput")
    id_in = nc.dram_tensor("ident", [BS, BS], FP16, kind="ExternalInput")
    z0t_in = nc.dram_tensor("z0t", [HID, BL], F32R, kind="ExternalInput")
    zs_out = nc.dram_tensor("zs", [HID, (nstep + 1) * BL], F32, kind="ExternalOutput")

    with TileContext(nc) as tc:
        with (
            tc.tile_pool(name="const", bufs=1) as cp,
            tc.tile_pool(name="zst", bufs=1) as zp,
            tc.tile_pool(name="hs", bufs=3) as hp,
            tc.tile_pool(name="fs", bufs=2) as fp,
            tc.tile_pool(name="us", bufs=2) as up,
            tc.tile_pool(name="ks", bufs=3) as kp,
            tc.tile_pool(name="an", bufs=2) as ap,
            tc.tile_pool(name="kh", bufs=2) as khp,
            tc.tile_pool(name="psA", bufs=1, space="PSUM") as psA,
            tc.tile_pool(name="psB", bufs=1, space="PSUM") as psB,
        ):
            b1S = cp.tile([MLP_H, nstep * 3], F32)
            gS = {}
            gS["A"] = cp.tile([BS, nstep * 3 * C_IN], FP16, name="gSA")
            gS["B"] = cp.tile([BS, nstep * 3 * C_IN], FP16, name="gSB")
            w1zS = cp.tile([HID, MLP_H], BD)
            w1zH = cp.tile([HID, MLP_H], FP16)
            w2S = cp.tile([MLP_H, NF], FP16)
            b2S = cp.tile([1, NF], FP16)
            onesS = cp.tile([1, BS], FP16)
            idS = cp.tile([BS, BS], FP16)
            zall = zp.tile([HID, (nstep + 1) * BL], F32R)

            nc.sync.dma_start(out=gS["A"][:], in_=gA_in[:])
            nc.sync.dma_start(out=gS["B"][:], in_=gB_in[:])
            nc.sync.dma_start(out=b1S[:], in_=b1_in[:])
            nc.sync.dma_start(out=w1zS[:], in_=w1z_in[:])
            nc.sync.dma_start(out=w1zH[:], in_=w1zh_in[:])
            nc.sync.dma_start(out=w2S[:], in_=w2_in[:])
            nc.sync.dma_start(out=b2S[:], in_=b2_in[:])
            nc.sync.dma_start(out=onesS[:], in_=ones_in[:])
            nc.sync.dma_start(out=idS[:], in_=id_in[:])
            nc.sync.dma_start(out=zall[:, 0:BL], in_=z0t_in[:])
            nc.sync.dma_start(out=zs_out[:, 0:BL], in_=z0t_in[:].bitcast(F32))

            # per-stream PSUM: one h bank (8 rotating [128,64] slots), one
            # f bank, one kt bank (4 rotating [64,64] slots: s=0..2 -> k^T,
            # s=3 -> acc^T). Slot rotation keeps at most one open
            # accumulation group per bank at a time.
            hP = {"A": psA.tile([MLP_H, 8 * BS], F32, name="hPA"),
                  "B": psB.tile([MLP_H, 8 * BS], F32, name="hPB")}
            fP = {"A": psA.tile([BS, NF], F32, name="fPA"),
                  "B": psB.tile([BS, NF], F32, name="fPB")}
            kT = {"A": psA.tile([HID, 4 * BS], FP16, name="kTA"),
                  "B": psB.tile([HID, 4 * BS], FP16, name="kTB")}

            def h_slot(st, step, s):
                i = (4 * step + s) % 8
                return hP[st][:, i * BS : (i + 1) * BS]

            def kt_slot(st, s):
                return kT[st][:, s * BS : (s + 1) * BS]

            CLS = (0, 1, 1, 2)
            KH_A = (1.0 / 6.0, 0.5, 0.25, 0.5)
            SS = ("A", "B")

            acc_nat = {"A": None, "B": None}
            STREAMS = {"A": slice(0, BS), "B": slice(BS, BL)}

            def zT_sl(st, step):
                off = step * BL + (0 if st == "A" else BS)
                return zall[:, off : off + BS]

            # step 0: slice-0 bigs (start+stop; no k accumulate at s=0)
            for st in SS:
                nc.tensor.matmul(
                    h_slot(st, 0, 0), lhsT=w1zS[:], rhs=zT_sl(st, 0),
                    start=True, stop=True,
                )

            for step in range(nstep):
                for s in range(4):
                    col = step * 3 + CLS[s]
                    has_b = not (step == 0 and s == 0)
                    # ---- front: kh, z-update, mm1acc, next big ----
                    for st in SS:
                        if has_b:
                            ksrc = kt_slot(st, 3) if s == 0 else kt_slot(st, s - 1)
                            kh = khp.tile([HID, BS], FP16, tag="kh" + st,
                                          name="kh" + st)
                            nc.vector.tensor_scalar_mul(kh[:], ksrc, KH_A[s])
                            if s == 0 and step > 0:
                                nc.vector.scalar_tensor_tensor(
                                    out=zT_sl(st, step),
                                    in0=ksrc,
                                    scalar=1.0 / 6.0,
                                    in1=zT_sl(st, step - 1),
                                    op0=mybir.AluOpType.mult,
                                    op1=mybir.AluOpType.add,
                                )
                                if st == "B":
                                    nc.sync.dma_start(
                                        out=zs_out[:, step * BL : (step + 1) * BL],
                                        in_=zall[
                                            :, step * BL : (step + 1) * BL
                                        ].bitcast(F32),
                                    )
                            nc.tensor.matmul(
                                h_slot(st, step, s), lhsT=w1zH[:], rhs=kh[:],
                                start=False, stop=True,
                            )
                        # big mm1 for next stage's slot (off critical path)
                        if s < 3:
                            nc.tensor.matmul(
                                h_slot(st, step, s + 1), lhsT=w1zS[:],
                                rhs=zT_sl(st, step), start=True, stop=False,
                            )
                        elif step + 1 < nstep:
                            nc.tensor.matmul(
                                h_slot(st, step + 1, 0), lhsT=w1zS[:],
                                rhs=zT_sl(st, step), start=True, stop=False,
                            )
                    # ---- relu + mm2 ----
                    hS = {}
                    for st in SS:
                        hS[st] = hp.tile([MLP_H, BS], FP16, tag="hs" + st,
                                         name="hs" + st)
                        if r_act:
                            nc.scalar.activation(
                                hS[st][:], h_slot(st, step, s),
                                mybir.ActivationFunctionType.Relu,
                                bias=b1S[:, col : col + 1],
                            )
                        else:
                            nc.vector.tensor_scalar(
                                hS[st][:], h_slot(st, step, s),
                                b1S[:, col : col + 1], 0.0,
                                op0=mybir.AluOpType.add,
                                op1=mybir.AluOpType.max,
                            )
                        if with_b2:
                            nc.tensor.matmul(
                                fP[st][:], lhsT=onesS[:], rhs=b2S[:],
                                start=True, stop=False,
                            )
                        nc.tensor.matmul(
                            fP[st][:], lhsT=hS[st][:], rhs=w2S[:],
                            start=not with_b2, stop=True,
                        )
                    # ---- tanh ----
                    fS = {}
                    for st in SS:
                        fS[st] = fp.tile([BS, NF], FP16, tag="fs" + st,
                                         name="fs" + st)
                        nc.scalar.activation(
                            fS[st][:], fP[st][:],
                            mybir.ActivationFunctionType.Tanh,
                        )
                    # ---- back: mul, reduce, RK4 sum, transpose ----
                    for st in SS:
                        u = up.tile([BS, NF], FP16, tag="u" + st, name="u" + st)
                        f3 = fS[st][:].rearrange("p (h c) -> p h c", c=C_IN)
                        u3 = u[:].rearrange("p (h c) -> p h c", c=C_IN)
                        gv = (
                            gS[st][:, col * C_IN : (col + 1) * C_IN]
                            .unsqueeze(1)
                            .broadcast_to((BS, HID, C_IN))
                        )
                        nc.vector.tensor_tensor(
                            out=u3, in0=f3, in1=gv, op=mybir.AluOpType.mult
                        )
                        kn = kp.tile([BS, HID], FP16, tag="kn" + st,
                                     name="kn" + st)
                        with nc.allow_low_precision("k reduce output precision"):
                            nc.vector.tensor_reduce(
                                kn[:], u3, axis=mybir.AxisListType.X,
                                op=mybir.AluOpType.add,
                            )
                        # RK4 sum in natural layout (ping-pong adds on DVE)
                        if s == 0:
                            acc_nat[st] = kn
                        else:
                            acc_new = ap.tile([BS, HID], FP16, tag="an" + st,
                                              name="an" + st)
                            nc.vector.tensor_tensor(
                                out=acc_new[:], in0=acc_nat[st][:], in1=kn[:],
                                op=mybir.AluOpType.add,
                            )
                            acc_nat[st] = acc_new
                        # transpose: k^T (s<3) or acc^T (s=3), fp16 1-pass
                        nc.tensor.matmul(
                            kt_slot(st, s),
                            lhsT=(kn[:] if s < 3 else acc_nat[st][:]),
                            rhs=idS[:], is_transpose=True,
                            start=True, stop=True,
                        )

            # final z update (last grid point)
            for st in SS:
                nc.vector.scalar_tensor_tensor(
                    out=zT_sl(st, nstep),
                    in0=kt_slot(st, 3),
                    scalar=1.0 / 6.0,
                    in1=zT_sl(st, nstep - 1),
                    op0=mybir.AluOpType.mult,
                    op1=mybir.AluOpType.add,
                )
            nc.sync.dma_start(
                out=zs_out[:, nstep * BL : (nstep + 1) * BL],
                in_=zall[:, nstep * BL : (nstep + 1) * BL].bitcast(F32),
            )
    import sys

    print(f"[kernel] tile trace+schedule: {_time.time()-t0:.1f}s", file=sys.stderr)
    t1 = _time.time()
    nc.finalize()
    print(f"[kernel] finalize: {_time.time()-t1:.1f}s", file=sys.stderr)
    return nc


def _get_nc(nstep: int, with_b2: bool):
    key = (nstep, with_b2) + _flags()
    if key not in _CACHE:
        _CACHE[key] = _build(nstep, with_b2)
    return _CACHE[key]


def _host_prep(coeffs, Wi1, bi1, Wi2, bi2, W1, b1, W2, b2, nstep: int):
    coeffs = np.asarray(coeffs, dtype=np.float32)
    a = coeffs[:, :, 0:8]
    b = coeffs[:, :, 8:16]
    c = coeffs[:, :, 16:24]
    d = coeffs[:, :, 24:32]

    X0 = a[:, 0]
    z0 = np.tanh(
        np.maximum(X0 @ Wi1 + bi1, 0.0).astype(np.float32) @ Wi2 + bi2
    ).astype(np.float32)

    g = np.empty((B, nstep, 3, C_IN), dtype=np.float32)
    g[:, :, 0] = b[:, :nstep]
    g[:, :, 1] = 2.0 * b[:, :nstep] + 2.0 * c[:, :nstep] + 1.5 * d[:, :nstep]
    # stage-4 derivative: dXdt at t=i+1
    last = NSTEP - 1  # 126 in full problem
    for i in range(nstep):
        if i < last:
            g[:, i, 2] = b[:, i + 1]
        else:
            g[:, i, 2] = b[:, i] + 2.0 * c[:, i] + 3.0 * d[:, i]
    g16 = g.reshape(B, nstep * 3 * C_IN).astype(np.float16)

    tcols = np.empty((nstep, 3), dtype=np.float32)
    tcols[:, 0] = np.arange(nstep, dtype=np.float32)
    tcols[:, 1] = tcols[:, 0] + 0.5
    tcols[:, 2] = tcols[:, 0] + 1.0
    # bias1[m, step*3+cls] = b1[m] + t * W1[0, m]
    bias1 = (
        b1[None, None, :] + tcols[:, :, None] * W1[0][None, None, :]
    ).astype(np.float32)
    bias1 = bias1.reshape(nstep * 3, MLP_H).T.copy()  # [128, nstep*3]

    f16_big = _flags()[0]
    shared = {
        "bias1": bias1,
        "w1z": np.ascontiguousarray(
            W1[1:], dtype=(np.float16 if f16_big else np.float32)
        ),
        "w1zh": np.ascontiguousarray(W1[1:], dtype=np.float16),
        "w2": np.ascontiguousarray(W2, dtype=np.float16),
        "b2r": np.ascontiguousarray(b2[None, :], dtype=np.float16),
        "onesr": np.ones((1, BS), dtype=np.float16),
        "ident": np.eye(BS, dtype=np.float16),
    }
    in_maps = []
    for core in range(NCORES):
        sl = slice(core * BL, (core + 1) * BL)
        m = dict(shared)
        gc = g16[sl]
        m["gA"] = np.ascontiguousarray(gc[0:BS])
        m["gB"] = np.ascontiguousarray(gc[BS:BL])
        m["z0t"] = np.ascontiguousarray(z0[sl].T)
        in_maps.append(m)
    return in_maps, z0


def kernel(coeffs, Wi1, bi1, Wi2, bi2, W1, b1, W2, b2, _nstep: int = NSTEP,
           _trace: bool = False):
    import time as _time
    import sys

    nstep = _nstep
    with_b2 = bool(np.any(np.asarray(b2)))
    nc = _get_nc(nstep, with_b2)
    in_maps, _ = _host_prep(
        coeffs, Wi1, bi1, Wi2, bi2, W1, b1, W2, b2, nstep
    )
    t0 = _time.time()
    res = run_bass_kernel_spmd(nc, in_maps, list(range(NCORES)), trace=_trace)
    print(f"[kernel] spmd run (compile+exec): {_time.time()-t0:.1f}s", file=sys.stderr)
    out = np.empty((B, nstep + 1, HID), dtype=np.float32)
    for core in range(NCORES):
        zs = res.results[core]["zs"].reshape(HID, nstep + 1, BL)
        out[core * BL : (core + 1) * BL] = zs.transpose(2, 1, 0)
    if _trace:
        kernel.last_results = res
    return out


# revision 18
# speedup vs baseline: 1.2575x; 1.2371x over previous
"""NeuralCDE RK4 solver as a Bass/Tile kernel on 8 Trainium2 cores.

Data-parallel over batch: B=1024 -> 128 rows per core (one partition tile).
Wall time = 508 serial RK4 stages x per-stage chain latency, so everything
here is about shortening that chain:
    kh    (DVE) : alpha*k^T   fp16 PSUM -> SBUF fp16 (2x mode)
    mm1acc (PE) : h_ps slot += W1zH.T @ kh    (big W1z.T @ zT pre-issued)
    relu  (DVE) : hS = relu(h_ps + bias1(t))
    mm2   (PE)  : f_ps = hS.T @ W2, split into two 256-col halves
    tanh  (ACT) : per half -> overlaps DVE mul/reduce of the other half
    mul   (DVE) : u = fS * g (broadcast over h), per half
    red   (DVE) : kn half = sum_c u
    T     (PE)  : kn^T -> ksP (fp16 non-accumulating, 1-pass)
RK4 sum: kn adds on DVE (off-chain); at s=3 the partial sum transposes
off-chain through a (1/6)-scaled identity, and one scalar_tensor_tensor
yields delta-z^T for both the state update and next step's h correction.
PE runs at the cold 1.2 GHz HAM rate (filler matmuls do not lift the clock
gate in this environment - verified experimentally).
"""

import numpy as np
import ml_dtypes

import concourse.bacc as bacc
import concourse.bass as bass
import concourse.mybir as mybir
from concourse.tile import TileContext
from concourse.bass_utils import run_bass_kernel_spmd

F32 = mybir.dt.float32
F32R = mybir.dt.float32r
BF16 = mybir.dt.bfloat16
FP16 = mybir.dt.float16
B = 1024
L = 128
C_IN = 8
HID = 64
MLP_H = 128
INIT_H = 20
NSTEP = L - 1  # 127
NCORES = 8
BL = B // NCORES  # 128 batch rows per core
NF = HID * C_IN  # 512
NH = NF // 2  # 256 (half of the f block, h-split)

_CACHE: dict = {}


def _flags():
    import os
    return (
        int(os.environ.get("F16_BIG", "0")),   # big mm1 in fp16 instead of f32r
        int(os.environ.get("SPLIT", "1")),     # half-split the f block
    )


def _build(nstep: int, with_b2: bool):
    import time as _time

    f16_big, split = _flags()
    BD = FP16 if f16_big else F32R
    t0 = _time.time()
    nc = bacc.Bacc()
    g_in = nc.dram_tensor("g", [BL, nstep * 3 * C_IN], FP16, kind="ExternalInput")
    b1_in = nc.dram_tensor("bias1", [MLP_H, nstep * 3], F32, kind="ExternalInput")
    w1z_in = nc.dram_tensor("w1z", [HID, MLP_H], BD, kind="ExternalInput")
    w1zh_in = nc.dram_tensor("w1zh", [HID, MLP_H], FP16, kind="ExternalInput")
    w2_in = nc.dram_tensor("w2", [MLP_H, NF], FP16, kind="ExternalInput")
    b2_in = nc.dram_tensor("b2r", [1, NF], FP16, kind="ExternalInput")
    ones_in = nc.dram_tensor("onesr", [1, BL], FP16, kind="ExternalInput")
    id_in = nc.dram_tensor("ident", [BL, BL], FP16, kind="ExternalInput")
    id6_in = nc.dram_tensor("ident6", [BL, BL], FP16, kind="ExternalInput")
    z0t_in = nc.dram_tensor("z0t", [HID, BL], F32R, kind="ExternalInput")
    zs_out = nc.dram_tensor("zs", [HID, (nstep + 1) * BL], F32, kind="ExternalOutput")

    with TileContext(nc) as tc:
        with (
            tc.tile_pool(name="const", bufs=1) as cp,
            tc.tile_pool(name="zst", bufs=1) as zp,
            tc.tile_pool(name="hs", bufs=3) as hp,
            tc.tile_pool(name="fs", bufs=3) as fp,
            tc.tile_pool(name="us", bufs=3) as up,
            tc.tile_pool(name="ks", bufs=3) as kp,
            tc.tile_pool(name="an", bufs=2) as ap,
            tc.tile_pool(name="zs2", bufs=2) as zsp,
            tc.tile_pool(name="kh", bufs=2) as khp,
            tc.tile_pool(name="ph", bufs=1, space="PSUM") as ph,
            tc.tile_pool(name="pf", bufs=2, space="PSUM") as pf,
            tc.tile_pool(name="pkt", bufs=3, space="PSUM") as pkt,
            tc.tile_pool(name="pa26", bufs=1, space="PSUM") as pa26,
        ):
            b1S = cp.tile([MLP_H, nstep * 3], F32)
            gS = cp.tile([BL, nstep * 3 * C_IN], FP16)
            w1zS = cp.tile([HID, MLP_H], BD)
            w1zH = cp.tile([HID, MLP_H], FP16)
            w2S = cp.tile([MLP_H, NF], FP16)
            b2S = cp.tile([1, NF], FP16)
            onesS = cp.tile([1, BL], FP16)
            idS = cp.tile([BL, BL], FP16)
            id6S = cp.tile([BL, BL], FP16)
            zall = zp.tile([HID, (nstep + 1) * BL], F32R)

            nc.sync.dma_start(out=gS[:], in_=g_in[:])
            nc.sync.dma_start(out=b1S[:], in_=b1_in[:])
            nc.sync.dma_start(out=w1zS[:], in_=w1z_in[:])
            nc.sync.dma_start(out=w1zH[:], in_=w1zh_in[:])
            nc.sync.dma_start(out=w2S[:], in_=w2_in[:])
            nc.sync.dma_start(out=b2S[:], in_=b2_in[:])
            nc.sync.dma_start(out=onesS[:], in_=ones_in[:])
            nc.sync.dma_start(out=idS[:], in_=id_in[:])
            nc.sync.dma_start(out=id6S[:], in_=id6_in[:])
            nc.sync.dma_start(out=zall[:, 0:BL], in_=z0t_in[:])
            nc.sync.dma_start(out=zs_out[:, 0:BL], in_=z0t_in[:].bitcast(F32))

            # h PSUM: one bank, 4 rotating [128,128] stage slots. Emission
            # order guarantees at most one open accumulation group at a time.
            hP = ph.tile([MLP_H, 4 * BL], F32, name="hP")

            def h_slot(step, s):
                i = (4 * step + s) % 4
                return hP[:, i * BL : (i + 1) * BL]

            CLS = (0, 1, 1, 2)
            KH_A = (1.0 / 6.0, 0.5, 0.25, 0.5)

            def zT_sl(step):
                return zall[:, step * BL : (step + 1) * BL]

            # step 0 slice-0 big (no k correction at the very first stage)
            nc.tensor.matmul(
                h_slot(0, 0), lhsT=w1zS[:], rhs=zT_sl(0), start=True, stop=True
            )

            acc_nat = None   # kn1+kn2 (+kn3) natural-layout partial RK4 sum
            acc2T6 = None    # (acc_nat at s=2).T / 6 in PSUM
            zsum = None      # zT + acc2T6, f32r (state update staging)
            kt4P = None      # k4~.T PSUM
            ksP = None       # k~_s.T PSUM for next stage's kh
            kh0 = None       # delta-z^T fp16 (next step's h correction)

            for step in range(nstep):
                zT = zT_sl(step)
                for s in range(4):
                    col = step * 3 + CLS[s]
                    has_b = not (step == 0 and s == 0)
                    # ---- kh for this stage ----
                    if has_b:
                        kh = khp.tile([HID, BL], FP16, tag="kh", name="kh")
                        if s == 0:
                            # kh0 = (k4~.T)/6 + acc2T6 = delta-z^T
                            nc.vector.scalar_tensor_tensor(
                                out=kh[:],
                                in0=kt4P[:],
                                scalar=1.0 / 6.0,
                                in1=acc2T6[:],
                                op0=mybir.AluOpType.mult,
                                op1=mybir.AluOpType.add,
                            )
                            # state update z_step = zsum + (k4~.T)/6, f32r
                            nc.vector.scalar_tensor_tensor(
                                out=zT,
                                in0=kt4P[:],
                                scalar=1.0 / 6.0,
                                in1=zsum[:],
                                op0=mybir.AluOpType.mult,
                                op1=mybir.AluOpType.add,
                            )
                            nc.sync.dma_start(
                                out=zs_out[:, step * BL : (step + 1) * BL],
                                in_=zT.bitcast(F32),
                            )
                        else:
                            nc.vector.tensor_scalar_mul(kh[:], ksP[:], KH_A[s])
                        nc.tensor.matmul(
                            h_slot(step, s), lhsT=w1zH[:], rhs=kh[:],
                            start=False, stop=True,
                        )
                    # ---- big mm1 for the next stage slot (off chain) ----
                    if s < 3:
                        nc.tensor.matmul(
                            h_slot(step, s + 1), lhsT=w1zS[:], rhs=zT,
                            start=True, stop=False,
                        )
                    elif step + 1 < nstep:
                        nc.tensor.matmul(
                            h_slot(step + 1, 0), lhsT=w1zS[:], rhs=zT,
                            start=True, stop=False,
                        )
                    # ---- relu ----
                    hS = hp.tile([MLP_H, BL], FP16, tag="hs")
                    nc.vector.tensor_scalar(
                        hS[:], h_slot(step, s), b1S[:, col : col + 1], 0.0,
                        op0=mybir.AluOpType.add, op1=mybir.AluOpType.max,
                    )
                    # ---- mm2 (+ optional bias2), h-split halves ----
                    f_ps = pf.tile([BL, NF], F32, tag="fps")
                    if with_b2:
                        nc.tensor.matmul(
                            f_ps[:], lhsT=onesS[:], rhs=b2S[:],
                            start=True, stop=False,
                        )
                    halves = ((0, NH), (NH, NF)) if split else ((0, NF),)
                    for (lo, hi) in halves:
                        nc.tensor.matmul(
                            f_ps[:, lo:hi], lhsT=hS[:], rhs=w2S[:, lo:hi],
                            start=not with_b2, stop=True,
                        )
                    # ---- tanh / mul / reduce, pipelined across halves ----
                    fS = fp.tile([BL, NF], FP16, tag="fs")
                    u = up.tile([BL, NF], FP16, tag="u")
                    kn = kp.tile([BL, HID], FP16, tag="kn")
                    for (lo, hi) in halves:
                        nc.scalar.activation(
                            fS[:, lo:hi], f_ps[:, lo:hi],
                            mybir.ActivationFunctionType.Tanh,
                        )
                    for (lo, hi) in halves:
                        hlo, hhi = lo // C_IN, hi // C_IN
                        f3 = fS[:, lo:hi].rearrange("p (h c) -> p h c", c=C_IN)
                        u3 = u[:, lo:hi].rearrange("p (h c) -> p h c", c=C_IN)
                        gv = (
                            gS[:, col * C_IN : (col + 1) * C_IN]
                            .unsqueeze(1)
                            .broadcast_to((BL, hhi - hlo, C_IN))
                        )
                        nc.vector.tensor_tensor(
                            out=u3, in0=f3, in1=gv, op=mybir.AluOpType.mult
                        )
                        with nc.allow_low_precision("k reduce"):
                            nc.vector.tensor_reduce(
                                kn[:, hlo:hhi], u3, axis=mybir.AxisListType.X,
                                op=mybir.AluOpType.add,
                            )
                    # ---- transpose + RK4 bookkeeping ----
                    if s < 3:
                        ksP = pkt.tile([HID, BL], FP16, tag="kt", name="kt")
                        nc.tensor.matmul(
                            ksP[:], lhsT=kn[:], rhs=idS[:], is_transpose=True,
                            start=True, stop=True,
                        )
                        if s == 0:
                            acc_nat = kn
                        else:
                            acc_new = ap.tile([BL, HID], FP16, tag="an",
                                              name="an")
                            nc.vector.tensor_tensor(
                                out=acc_new[:], in0=acc_nat[:], in1=kn[:],
                                op=mybir.AluOpType.add,
                            )
                            acc_nat = acc_new
                        if s == 2:
                            # off-chain: acc2T6 = acc_nat.T / 6 (via scaled
                            # identity), staged to SBUF so the s=0 STT has a
                            # single PSUM operand; zsum = zT + acc2T6
                            a26P = pa26.tile([HID, BL], F32, tag="a26",
                                             name="a26")
                            # regular matmul: transpose datapath would ignore
                            # the scaled identity's values
                            nc.tensor.matmul(
                                a26P[:], lhsT=acc_nat[:], rhs=id6S[:],
                                start=True, stop=True,
                            )
                            acc2T6 = zsp.tile([HID, BL], FP16, tag="a26s",
                                              name="a26s")
                            nc.vector.tensor_copy(acc2T6[:], a26P[:])
                            zsum = zsp.tile([HID, BL], F32R, tag="zsum",
                                            name="zsum")
                            nc.vector.tensor_tensor(
                                out=zsum[:], in0=zT, in1=acc2T6[:],
                                op=mybir.AluOpType.add,
                            )
                    else:
                        kt4P = pkt.tile([HID, BL], FP16, tag="kt", name="kt4")
                        nc.tensor.matmul(
                            kt4P[:], lhsT=kn[:], rhs=idS[:], is_transpose=True,
                            start=True, stop=True,
                        )

            # final z update (last grid point)
            nc.vector.scalar_tensor_tensor(
                out=zT_sl(nstep),
                in0=kt4P[:],
                scalar=1.0 / 6.0,
                in1=zsum[:],
                op0=mybir.AluOpType.mult,
                op1=mybir.AluOpType.add,
            )
            nc.sync.dma_start(
                out=zs_out[:, nstep * BL : (nstep + 1) * BL],
                in_=zT_sl(nstep).bitcast(F32),
            )
    import sys

    print(f"[kernel] tile trace+schedule: {_time.time()-t0:.1f}s", file=sys.stderr)
    t1 = _time.time()
    nc.finalize()
    print(f"[kernel] finalize: {_time.time()-t1:.1f}s", file=sys.stderr)
    return nc


def _get_nc(nstep: int, with_b2: bool):
    key = (nstep, with_b2) + _flags()
    if key not in _CACHE:
        _CACHE[key] = _build(nstep, with_b2)
    return _CACHE[key]


def _host_prep(coeffs, Wi1, bi1, Wi2, bi2, W1, b1, W2, b2, nstep: int):
    coeffs = np.asarray(coeffs, dtype=np.float32)
    a = coeffs[:, :, 0:8]
    b = coeffs[:, :, 8:16]
    c = coeffs[:, :, 16:24]
    d = coeffs[:, :, 24:32]

    X0 = a[:, 0]
    z0 = np.tanh(
        np.maximum(X0 @ Wi1 + bi1, 0.0).astype(np.float32) @ Wi2 + bi2
    ).astype(np.float32)

    g = np.empty((B, nstep, 3, C_IN), dtype=np.float32)
    g[:, :, 0] = b[:, :nstep]
    g[:, :, 1] = 2.0 * b[:, :nstep] + 2.0 * c[:, :nstep] + 1.5 * d[:, :nstep]
    last = NSTEP - 1  # 126 in full problem
    for i in range(nstep):
        if i < last:
            g[:, i, 2] = b[:, i + 1]
        else:
            g[:, i, 2] = b[:, i] + 2.0 * c[:, i] + 3.0 * d[:, i]
    g16 = g.reshape(B, nstep * 3 * C_IN).astype(np.float16)

    tcols = np.empty((nstep, 3), dtype=np.float32)
    tcols[:, 0] = np.arange(nstep, dtype=np.float32)
    tcols[:, 1] = tcols[:, 0] + 0.5
    tcols[:, 2] = tcols[:, 0] + 1.0
    bias1 = (
        b1[None, None, :] + tcols[:, :, None] * W1[0][None, None, :]
    ).astype(np.float32)
    bias1 = bias1.reshape(nstep * 3, MLP_H).T.copy()  # [128, nstep*3]

    f16_big = _flags()[0]
    shared = {
        "bias1": bias1,
        "w1z": np.ascontiguousarray(
            W1[1:], dtype=(np.float16 if f16_big else np.float32)
        ),
        "w1zh": np.ascontiguousarray(W1[1:], dtype=np.float16),
        "w2": np.ascontiguousarray(W2, dtype=np.float16),
        "b2r": np.ascontiguousarray(b2[None, :], dtype=np.float16),
        "onesr": np.ones((1, BL), dtype=np.float16),
        "ident": np.eye(BL, dtype=np.float16),
        "ident6": (np.eye(BL) / 6.0).astype(np.float16),
    }
    in_maps = []
    for core in range(NCORES):
        sl = slice(core * BL, (core + 1) * BL)
        m = dict(shared)
        m["g"] = np.ascontiguousarray(g16[sl])
        m["z0t"] = np.ascontiguousarray(z0[sl].T)
        in_maps.append(m)
    return in_maps, z0


def kernel(coeffs, Wi1, bi1, Wi2, bi2, W1, b1, W2, b2, _nstep: int = NSTEP,
           _trace: bool = False):
    import time as _time
    import sys

    nstep = _nstep
    with_b2 = bool(np.any(np.asarray(b2)))
    nc = _get_nc(nstep, with_b2)
    in_maps, _ = _host_prep(
        coeffs, Wi1, bi1, Wi2, bi2, W1, b1, W2, b2, nstep
    )
    t0 = _time.time()
    res = run_bass_kernel_spmd(nc, in_maps, list(range(NCORES)), trace=_trace)
    print(f"[kernel] spmd run (compile+exec): {_time.time()-t0:.1f}s", file=sys.stderr)
    out = np.empty((B, nstep + 1, HID), dtype=np.float32)
    for core in range(NCORES):
        zs = res.results[core]["zs"].reshape(HID, nstep + 1, BL)
        out[core * BL : (core + 1) * BL] = zs.transpose(2, 1, 0)
    if _trace:
        kernel.last_results = res
    return out
